# revision 12
# baseline (speedup 1.0000x reference)
"""Trainium2 Bass kernel for nn_DiffusionActionHead (DDPM sampling head).

Strategy
--------
Pure data parallel over 8 NeuronCores: batch (32768) sharded to 4096 rows
per core; the ~1MB of MLP weights and schedule constants are replicated.

All activations live transposed in SBUF: [feature (partitions), batch
(free dim)], so every linear layer is out.T = W.T-chunks (stationary)
@ act.T (moving), PE-friendly with N=512 moving columns per matmul and
fp32 PSUM accumulation.  Matmuls run in float32r (FP22 mantissa-truncated
reads) which streams at 1 elem/cycle vs 4x slower true fp32.

Algebraic restructure (all host-side, tiny):
 - cond = features @ cond_w + cond_b is only consumed through
   blk0_w1/blk0_skip_w.  Fold: a1 = features @ (cond_w @ U1) and
   askip = features @ (cond_w @ Us) are computed once on device and kept
   resident; per-step they are accumulated into PSUM with an identity
   matmul.  (U1/Us = rows 0:256 of blk0_w1 / blk0_skip_w.)
 - The time embedding is a per-step scalar path: temb_t @ T1 + b1 (+
   cond_b @ U1) becomes a per-step bias vector riding an augmented
   ones-row on the x-state matmul (K=4 -> K=5), so it costs nothing.
 - x-state recurrence x' = c2*(x - c1*pred) + c3*n is rescaled by
   gamma_{i+1} = c2_i*gamma_i so the device update is a single add:
   y' = y + (h @ fw_hat_t + noise/bias terms), with fw_hat_t, the noise
   scale and final_b folded into per-step weights host-side.  Output is
   gamma_10 * y, clipped, on host.
"""

import math

import numpy as np

B, D, H, A = 32768, 256, 256, 4
STEPS, INF = 100, 10
NCORES = 8
BL = B // NCORES      # 4096 rows per core
NB = 512              # batch columns per tile (one fp32 PSUM bank)
NT = BL // NB         # 8 batch tiles per core
CAT = 2 * H + A


def _erf_vec(x):
    return np.array([math.erf(float(v)) for v in x.ravel()], dtype=np.float64).reshape(x.shape)


def _gelu_exact(x):
    x64 = x.astype(np.float64)
    return (0.5 * x64 * (1.0 + _erf_vec(x64 / math.sqrt(2.0)))).astype(np.float32)


def _pack_w(w):
    """[256, 256] weight -> [128, 512] lhsT tile; col block k*256+m*128
    holds W[k*128:(k+1)*128, m*128:(m+1)*128] (partition = contraction)."""
    assert w.shape == (256, 256)
    return np.ascontiguousarray(
        w.reshape(2, 128, 256).transpose(1, 0, 2).reshape(128, 512)
    ).astype(np.float32)


def _host_prep(inputs):
    f32 = np.float32
    g = {k: np.asarray(v, dtype=f32) for k, v in inputs.items()}

    blk0_w1, blk0_skip_w = g["blk0_w1"], g["blk0_skip_w"]
    U1, T1, X1 = blk0_w1[:256], blk0_w1[256:512], blk0_w1[512:516]
    Us, Ts, Xs = blk0_skip_w[:256], blk0_skip_w[256:512], blk0_skip_w[512:516]

    # diffusion schedule (float64 internally; applied as fp32-sized scalars)
    betas = np.linspace(1e-4, 0.02, STEPS).astype(np.float64)
    alphas = 1.0 - betas
    acp = np.cumprod(alphas)
    t_vals = np.arange(0, STEPS, max(1, STEPS // INF))[::-1]  # [90, 80, ..., 0]
    c1 = np.array([betas[t] / math.sqrt(1.0 - acp[t]) for t in t_vals])
    c2 = np.array([1.0 / math.sqrt(alphas[t]) for t in t_vals])
    c3 = np.array([math.sqrt(betas[t]) if t > 0 else 0.0 for t in t_vals])
    gamma = np.ones(INF + 1, dtype=np.float64)
    for i in range(INF):
        gamma[i + 1] = c2[i] * gamma[i]

    # time embedding path (per step, [256] vectors)
    half = H // 2
    freqs = np.exp(np.arange(half, dtype=f32) * f32(-math.log(10000.0) / (half - 1)))
    cb_U1 = g["cond_b"] @ U1
    cb_Us = g["cond_b"] @ Us
    augp = np.zeros((INF, 5, 512), dtype=f32)
    fwp = np.zeros((128, INF * 8), dtype=f32)
    nlw = np.zeros((5, INF * 4), dtype=f32)
    for i, t in enumerate(t_vals):
        e = f32(t) * freqs
        se = np.concatenate([np.sin(e), np.cos(e)]).astype(f32)
        temb = _gelu_exact(se @ g["time_w"] + g["time_b"])
        bias1 = temb @ T1 + g["blk0_b1"] + cb_U1
        biasS = temb @ Ts + g["blk0_skip_b"] + cb_Us
        augp[i, 0:4, 0:256] = f32(gamma[i]) * X1
        augp[i, 4, 0:256] = bias1
        augp[i, 0:4, 256:512] = f32(gamma[i]) * Xs
        augp[i, 4, 256:512] = biasS
        s_pred = f32(-c1[i] * c2[i] / gamma[i + 1])
        for k in range(2):
            fwp[:, i * 8 + k * 4:i * 8 + (k + 1) * 4] = \
                g["final_w"][k * 128:(k + 1) * 128] * s_pred
        nlw[0:4, i * 4:(i + 1) * 4] = np.eye(4, dtype=f32) * f32(c3[i] / gamma[i + 1])
        nlw[4, i * 4:(i + 1) * 4] = g["final_b"] * s_pred

    shared = {
        "a1w": _pack_w(g["cond_w"] @ U1),
        "asw": _pack_w(g["cond_w"] @ Us),
        "w2p": _pack_w(g["blk0_w2"]),
        "wb": np.concatenate(
            [_pack_w(w) for j in range(3) for w in (g["blks_w1"][j], g["blks_w2"][j])],
            axis=1,
        ),  # [128, 3072]
        "fwp": fwp,
        "augp": augp,
        "nlw": nlw,
        "biasp": np.stack(
            [g["blk0_b2"][0:128], g["blk0_b2"][128:256]]
            + [g[n][j][c * 128:(c + 1) * 128]
               for j in range(3) for n in ("blks_b1", "blks_b2") for c in range(2)],
            axis=1,
        ).astype(f32),  # [128, 14]: b2 c0/c1, then per block b1 c0/c1, b2 c0/c1
        "ident": np.eye(128, dtype=f32),
    }

    # deterministic diffusion noise, identical to the reference's jax PRNG
    import jax
    import jax.numpy as jnp
    with jax.default_device(jax.devices("cpu")[0]):
        nkey = jax.random.key(42)
        x0 = np.asarray(jax.random.normal(
            jax.random.fold_in(nkey, 999), (B, A), dtype=jnp.float32))
        noises = np.asarray(jax.random.normal(
            jax.random.fold_in(nkey, 7), (INF, B, A), dtype=jnp.float32))

    feats = g["features"]
    per_core = []
    for c in range(NCORES):
        sl = slice(c * BL, (c + 1) * BL)
        ftc = np.ascontiguousarray(feats[sl].T).reshape(2, 128, BL)
        xa = np.ones((5, BL), dtype=f32)
        xa[0:4] = x0[sl].T
        nz = np.ones((INF, NT, 5, NB), dtype=f32)
        for i in range(INF):
            nzt = noises[i, sl].T  # [4, BL]
            nz[i, :, 0:4, :] = nzt.reshape(4, NT, NB).transpose(1, 0, 2)
        per_core.append({"featsT": ftc, "xaug0": xa, "nzp": nz})
    return shared, per_core, np.float64(gamma[INF])


def _build_bass(pair=2, acts_bufs=8, h_bufs=5, nz_bufs=3, ps_bufs=None, IL=4):
    import concourse.mybir as mybir
    import concourse.tile as tile
    from concourse import bacc

    f32 = mybir.dt.float32
    f32r = mybir.dt.float32r
    GELU = mybir.ActivationFunctionType.Gelu
    ADD = mybir.AluOpType.add

    NBG = NB * pair          # batch columns per group
    NG = BL // NBG           # groups per core
    if ps_bufs is None:
        ps_bufs = 8 // pair  # 8 PSUM banks total

    nc = bacc.Bacc("TRN2", target_bir_lowering=False, debug=False,
                   num_devices=NCORES)

    d_feats = nc.dram_tensor("featsT", [2, 128, BL], f32r, kind="ExternalInput")
    d_a1w = nc.dram_tensor("a1w", [128, 512], f32r, kind="ExternalInput")
    d_asw = nc.dram_tensor("asw", [128, 512], f32r, kind="ExternalInput")
    d_w2p = nc.dram_tensor("w2p", [128, 512], f32r, kind="ExternalInput")
    d_wb = nc.dram_tensor("wb", [128, 3072], f32r, kind="ExternalInput")
    d_fwp = nc.dram_tensor("fwp", [128, INF * 8], f32r, kind="ExternalInput")
    d_augp = nc.dram_tensor("augp", [INF, 5, 512], f32r, kind="ExternalInput")
    d_nlw = nc.dram_tensor("nlw", [5, INF * 4], f32r, kind="ExternalInput")
    d_biasp = nc.dram_tensor("biasp", [128, 14], f32, kind="ExternalInput")
    d_ident = nc.dram_tensor("ident", [128, 128], f32r, kind="ExternalInput")
    d_xaug0 = nc.dram_tensor("xaug0", [5, BL], f32r, kind="ExternalInput")
    d_nzp = nc.dram_tensor("nzp", [INF, NT, 5, NB], f32r, kind="ExternalInput")
    d_out = nc.dram_tensor("out", [4, BL], f32r, kind="ExternalOutput")

    with tile.TileContext(nc) as tc:
        with tc.tile_pool(name="singles", bufs=1) as sg, \
             tc.tile_pool(name="stream", bufs=nz_bufs) as st, \
             tc.tile_pool(name="acts", bufs=acts_bufs) as ac, \
             tc.tile_pool(name="hbuf", bufs=h_bufs) as hb, \
             tc.tile_pool(name="psum", bufs=ps_bufs, space="PSUM") as ps:

            def mm(out, lhsT, rhs, start, stop):
                nc.tensor.matmul(out, lhsT, rhs, start=start, stop=stop)

            # --- resident constants -------------------------------------
            s_a1w = sg.tile([128, 512], f32r, tag="a1w")
            s_asw = sg.tile([128, 512], f32r, tag="asw")
            s_w2p = sg.tile([128, 512], f32r, tag="w2p")
            s_wb = sg.tile([128, 3072], f32r, tag="wb")
            s_fwp = sg.tile([128, INF * 8], f32r, tag="fwp")
            s_nlw = sg.tile([5, INF * 4], f32r, tag="nlw")
            s_biasp = sg.tile([128, 14], f32, tag="biasp")
            s_ident = sg.tile([128, 128], f32r, tag="ident")
            for dst, src in ((s_a1w, d_a1w), (s_asw, d_asw), (s_w2p, d_w2p),
                             (s_wb, d_wb), (s_fwp, d_fwp), (s_nlw, d_nlw),
                             (s_biasp, d_biasp), (s_ident, d_ident)):
                nc.sync.dma_start(out=dst[:], in_=src[:])

            # per-group x state (rows 0:4 = gamma-scaled x, row 4 = ones)
            xt = []
            for g in range(NG):
                x_g = sg.tile([5, NBG], f32r, tag=f"x{g}")
                nc.sync.dma_start(out=x_g[:], in_=d_xaug0[:, g * NBG:(g + 1) * NBG])
                xt.append(x_g)

            # --- setup: a1 / askip resident projections -----------------
            a1t = [[None] * NG for _ in range(2)]
            ast = [[None] * NG for _ in range(2)]
            for g in range(NG):
                cols = slice(g * NBG, (g + 1) * NBG)
                f0 = st.tile([128, NBG], f32r, tag="f0", bufs=2)
                f1 = st.tile([128, NBG], f32r, tag="f1", bufs=2)
                nc.sync.dma_start(out=f0[:], in_=d_feats[0, :, cols])
                nc.sync.dma_start(out=f1[:], in_=d_feats[1, :, cols])
                for m in range(2):
                    pa = ps.tile([128, NBG], f32, tag="mm")
                    pb = ps.tile([128, NBG], f32, tag="mm")
                    for hh in range(pair):
                        hc = slice(hh * NB, (hh + 1) * NB)
                        mm(pa[:, hc], s_a1w[:, m * 128:(m + 1) * 128],
                           f0[:, hc], True, False)
                        mm(pa[:, hc], s_a1w[:, 256 + m * 128:256 + (m + 1) * 128],
                           f1[:, hc], False, True)
                        mm(pb[:, hc], s_asw[:, m * 128:(m + 1) * 128],
                           f0[:, hc], True, False)
                        mm(pb[:, hc], s_asw[:, 256 + m * 128:256 + (m + 1) * 128],
                           f1[:, hc], False, True)
                    a1 = sg.tile([128, NBG], f32r, tag=f"a1_{m}_{g}")
                    nc.scalar.copy(out=a1[:], in_=pa[:])
                    a1t[m][g] = a1
                    as_ = sg.tile([128, NBG], f32r, tag=f"as_{m}_{g}")
                    nc.vector.tensor_copy(out=as_[:], in_=pb[:])
                    ast[m][g] = as_

            # --- main diffusion loop ------------------------------------
            # Per-(step, group) work is emitted as a phase generator and
            # round-robined across IL groups so each engine's in-order
            # stream interleaves independent chains (keeps ACT/PE/DVE all
            # busy instead of serializing on one chain's RAW deps).
            def chain(i, g, aug):
                nz = st.tile([5, NBG], f32r, tag="nz")
                for hh in range(pair):
                    nc.sync.dma_start(
                        out=nz[:, hh * NB:(hh + 1) * NB],
                        in_=d_nzp[i, g * pair + hh, :, :])
                yield

                # pre1 = a1 + x@ (gamma*X1) + bias-row
                p1 = []
                for m in range(2):
                    p = ps.tile([128, NBG], f32, tag="mm")
                    for hh in range(pair):
                        hc = slice(hh * NB, (hh + 1) * NB)
                        mm(p[:, hc], s_ident[:], a1t[m][g][:, hc], True, False)
                        mm(p[:, hc], aug[:, m * 128:(m + 1) * 128],
                           xt[g][:, hc], False, True)
                    p1.append(p)
                yield
                u = []
                for m in range(2):
                    ut = ac.tile([128, NBG], f32r, tag="gl")
                    nc.scalar.activation(out=ut[:], in_=p1[m][:], func=GELU)
                    u.append(ut)
                yield

                # pre2 = u @ blk0_w2  (+b2 in the gelu bias)
                p2 = []
                for m in range(2):
                    p = ps.tile([128, NBG], f32, tag="mm")
                    for hh in range(pair):
                        hc = slice(hh * NB, (hh + 1) * NB)
                        mm(p[:, hc], s_w2p[:, m * 128:(m + 1) * 128],
                           u[0][:, hc], True, False)
                        mm(p[:, hc], s_w2p[:, 256 + m * 128:256 + (m + 1) * 128],
                           u[1][:, hc], False, True)
                    p2.append(p)
                yield

                # skip = askip + x @ (gamma*Xs) + bias-row
                sk = []
                for m in range(2):
                    p = ps.tile([128, NBG], f32, tag="mm")
                    for hh in range(pair):
                        hc = slice(hh * NB, (hh + 1) * NB)
                        mm(p[:, hc], s_ident[:], ast[m][g][:, hc], True, False)
                        mm(p[:, hc], aug[:, 256 + m * 128:256 + (m + 1) * 128],
                           xt[g][:, hc], False, True)
                    sk.append(p)
                yield

                h = []
                for m in range(2):
                    vt = ac.tile([128, NBG], f32r, tag="gl")
                    nc.scalar.activation(out=vt[:], in_=p2[m][:], func=GELU,
                                         bias=s_biasp[:, m:m + 1], scale=1.0)
                    ht = hb.tile([128, NBG], f32r, tag=f"h{m}")
                    nc.vector.tensor_tensor(out=ht[:], in0=vt[:], in1=sk[m][:],
                                            op=ADD)
                    h.append(ht)
                yield

                # residual blocks 1..3
                for j in range(3):
                    w1o = j * 1024
                    w2o = j * 1024 + 512
                    bco = 2 + 4 * j
                    gt = []
                    for m in range(2):
                        p = ps.tile([128, NBG], f32, tag="mm")
                        for hh in range(pair):
                            hc = slice(hh * NB, (hh + 1) * NB)
                            mm(p[:, hc], s_wb[:, w1o + m * 128:w1o + (m + 1) * 128],
                               h[0][:, hc], True, False)
                            mm(p[:, hc],
                               s_wb[:, w1o + 256 + m * 128:w1o + 256 + (m + 1) * 128],
                               h[1][:, hc], False, True)
                        gg = ac.tile([128, NBG], f32r, tag="gl")
                        nc.scalar.activation(out=gg[:], in_=p[:], func=GELU,
                                             bias=s_biasp[:, bco + m:bco + m + 1],
                                             scale=1.0)
                        gt.append(gg)
                    yield
                    hn = []
                    for m in range(2):
                        p = ps.tile([128, NBG], f32, tag="mm")
                        for hh in range(pair):
                            hc = slice(hh * NB, (hh + 1) * NB)
                            mm(p[:, hc], s_wb[:, w2o + m * 128:w2o + (m + 1) * 128],
                               gt[0][:, hc], True, False)
                            mm(p[:, hc],
                               s_wb[:, w2o + 256 + m * 128:w2o + 256 + (m + 1) * 128],
                               gt[1][:, hc], False, True)
                        q = ac.tile([128, NBG], f32r, tag="gl")
                        nc.scalar.activation(out=q[:], in_=p[:], func=GELU,
                                             bias=s_biasp[:, bco + 2 + m:bco + 3 + m],
                                             scale=1.0)
                        ht = hb.tile([128, NBG], f32r, tag=f"h{m}")
                        nc.vector.tensor_tensor(out=ht[:], in0=q[:], in1=h[m][:],
                                                op=ADD)
                        hn.append(ht)
                    h = hn
                    yield

                # y += h @ fw_hat + (noise, final_b) via augmented lhsT
                pf = ps.tile([128, NBG], f32, tag="mm")
                for hh in range(pair):
                    hc = slice(hh * NB, (hh + 1) * NB)
                    mm(pf[0:4, hc], s_fwp[:, i * 8:i * 8 + 4], h[0][:, hc],
                       True, False)
                    mm(pf[0:4, hc], s_fwp[:, i * 8 + 4:i * 8 + 8], h[1][:, hc],
                       False, False)
                    mm(pf[0:4, hc], s_nlw[:, i * 4:(i + 1) * 4],
                       nz[:, hc], False, True)
                nc.vector.tensor_tensor(out=xt[g][0:4, :], in0=xt[g][0:4, :],
                                        in1=pf[0:4, :], op=ADD)
                yield

            # rolling window of IL chains across ALL (step, group) pairs —
            # no pipeline drain at step boundaries
            aug_tiles = {}

            def get_aug(i):
                if i not in aug_tiles:
                    a = st.tile([5, 512], f32r, tag="aug", bufs=3)
                    nc.sync.dma_start(out=a[:], in_=d_augp[i, :, :])
                    aug_tiles[i] = a
                return aug_tiles[i]

            seq = [(i, g) for i in range(INF) for g in range(NG)]
            active = []
            idx = 0
            while active or idx < len(seq):
                while len(active) < IL and idx < len(seq):
                    i, g = seq[idx]
                    idx += 1
                    active.append(chain(i, g, get_aug(i)))
                for ge in list(active):
                    try:
                        next(ge)
                    except StopIteration:
                        active.remove(ge)

            for g in range(NG):
                nc.sync.dma_start(out=d_out[:, g * NBG:(g + 1) * NBG],
                                  in_=xt[g][0:4, :])

    nc.finalize()
    return nc


_CACHE = {}


def kernel(**inputs) -> np.ndarray:
    from concourse.bass_utils import run_bass_kernel_spmd

    shared, per_core, gamma_final = _host_prep(inputs)
    if "nc" not in _CACHE:
        _CACHE["nc"] = _build_bass()
    nc = _CACHE["nc"]

    in_maps = []
    for c in range(NCORES):
        m = dict(shared)
        m.update(per_core[c])
        in_maps.append(m)
    res = None
    for attempt in range(3):
        try:
            res = run_bass_kernel_spmd(nc, in_maps, core_ids=list(range(NCORES)))
            break
        except Exception:
            # transient device wedge (NRT_EXEC_UNIT_UNRECOVERABLE) — retry
            if attempt == 2:
                raise
            import time as _time
            _time.sleep(45 + 30 * attempt)

    out = np.empty((B, A), dtype=np.float32)
    for c in range(NCORES):
        y = res.results[c]["out"]  # [4, BL]
        out[c * BL:(c + 1) * BL] = y.T
    out = np.clip(np.float32(gamma_final) * out, -1.0, 1.0).astype(np.float32)
    return out
